# revision 9
# baseline (speedup 1.0000x reference)
"""Trainium2 Bass kernel for nn_DistanceLoss (retrieval_knn, 5-way few-shot
temporal-tuple distance logits).

Math (per the reference):
  tuples = C(8,3) = 56 frame triples; embed dim 1024; supports grouped 5/class.
  qe = relu(q_tuples @ W.T + b); se likewise.
  logits[q,c] = -mean_t min_s ||qe[q,t] - se[c,s]||

Key restructuring on device:
  1. The 6144-wide gather-matmul is factorized: P[j,(a,f)] per frame (7x fewer
     FLOPs), then tuple-combine via run-grouped adds.  Frame columns are
     f-major and tuple columns t-major so every combine op streams contiguous
     50-element (query) / 25-element (support) inner runs.
  2. dist^2 = -2*(dot - q2/2 - s2/2): the norm terms are folded into the dots
     matmul as an extra K=2 accumulation chunk, so the class-min is a plain
     reduce_max over each class's PSUM block.
  3. mean over the 56 tuples is a [128,5].T @ [128,50] block-ones matmul.

Sharding: data-parallel over queries (50/core on 8 cores); support set and
weights replicated; host concatenates the per-core [5,50] outputs.
"""
import sys

sys.path.insert(0, '/opt/trn_rl_repo')
import numpy as np
import ml_dtypes
from itertools import combinations
from contextlib import ExitStack

from concourse import bass, bacc, tile, mybir
from concourse.bass_utils import run_bass_kernel_spmd

BF16 = ml_dtypes.bfloat16
F32 = mybir.dt.float32
BF = mybir.dt.bfloat16
RELU = mybir.ActivationFunctionType.Relu
SQRT = mybir.ActivationFunctionType.Sqrt

WAY, TSS = 5, 3
NS, NQ, SEQ, D = 25, 400, 8, 2048
D2 = 1024
NCORES = 8
QPC = NQ // NCORES              # 50 queries per core
SHOT = NS // WAY                # 5
TUP = list(combinations(range(SEQ), TSS))
TN = len(TUP)                   # 56
QT = QPC * TN                   # 2800 query-tuple columns per core
NMT = (QT + 127) // 128         # 22 M-tiles
QTP = NMT * 128                 # 2816 (padded)
STT = NS * TN                   # 1400 support-tuple columns
SPC = STT // WAY                # 280 per class
KC = D // 128                   # 16 contraction chunks for P
JCN = D2 // 128                 # 8 embed-dim chunks
FQ = QPC * SEQ                  # 400 query frame-columns
FS = NS * SEQ                   # 200 support frame-columns
F = FQ + FS                     # 600
WG = TSS * 128                  # 384 W columns per (kc, jc) group
NRUN = None

# runs of consecutive tuples sharing (f0, f1); within a run f2 is consecutive
RUNS = []
_t = 0
while _t < TN:
    f0, f1, f2 = TUP[_t]
    ln = 1
    while _t + ln < TN and TUP[_t + ln][0] == f0 and TUP[_t + ln][1] == f1:
        ln += 1
    RUNS.append((_t, ln, f0, f1, f2))
    _t += ln
NRUN = len(RUNS)                # 21


def build_nc():
    nc = bacc.Bacc("TRN2", target_bir_lowering=False, debug=False)
    # qt: [d, f*50+q], st: [d, f*25+s] (s class-sorted), w: [d, jc*384+a*128+jj]
    qt_d = nc.dram_tensor("qt", [D, FQ], BF, kind="ExternalInput")
    st_d = nc.dram_tensor("st", [D, FS], BF, kind="ExternalInput")
    w_d = nc.dram_tensor("w", [D, TSS * D2], BF, kind="ExternalInput")
    b_d = nc.dram_tensor("b", [128, JCN], F32, kind="ExternalInput")
    bm_d = nc.dram_tensor("bm", [128, NMT * QPC], BF, kind="ExternalInput")
    out_d = nc.dram_tensor("out", [WAY, QPC], F32, kind="ExternalOutput")

    with tile.TileContext(nc) as tc, ExitStack() as ctx:
        ep = ctx.enter_context
        wt_pool = ep(tc.tile_pool(name="wt", bufs=24))
        qst_pool = ep(tc.tile_pool(name="qst", bufs=KC))
        psb_pool = ep(tc.tile_pool(name="psb", bufs=2))
        pss_pool = ep(tc.tile_pool(name="pssb", bufs=4))
        s01_pool = ep(tc.tile_pool(name="s01", bufs=2))
        tmp_pool = ep(tc.tile_pool(name="tmp", bufs=2))
        qe_pool = ep(tc.tile_pool(name="qe", bufs=JCN))
        se_pool = ep(tc.tile_pool(name="se", bufs=JCN))
        sq_pool = ep(tc.tile_pool(name="sq", bufs=2))
        row_pool = ep(tc.tile_pool(name="row", bufs=1))
        misc_pool = ep(tc.tile_pool(name="misc", bufs=1))
        eps_pool = ep(tc.tile_pool(name="eps", bufs=3))

        # constants / persistent rows.
        # chunk9 K=2 operands: lhsT9 row0 = -q2/2 (bf16), row1 = ones;
        # rhs9 row0 = ones, row1 = -s2/2, so it contributes q2n[m] + s2n[n].
        # DVE compute must start at partition 0, so rows are computed in
        # 1-partition tiles and placed into partition 1 via SBUF->SBUF DMA.
        ones = misc_pool.tile([128, 1], BF, tag="ones")
        nc.vector.memset(ones[:], 1.0)
        ones_row = misc_pool.tile([1, QTP], BF, tag="ones_row")
        nc.vector.memset(ones_row[:], 1.0)
        b_sb = misc_pool.tile([128, JCN], F32, tag="bsb")
        nc.sync.dma_start(b_sb[:], b_d.ap())
        bm_sb = misc_pool.tile([128, NMT * QPC], BF, tag="bm")
        nc.sync.dma_start(bm_sb[:], bm_d.ap())
        lhsT9 = row_pool.tile([2, QTP], BF, tag="lhsT9")
        rhs9 = row_pool.tile([2, STT], BF, tag="rhs9")
        q2n = row_pool.tile([1, QTP], BF, tag="q2n")
        s2n = row_pool.tile([1, STT], BF, tag="s2n")
        q2r = row_pool.tile([1, QTP], F32, tag="q2r")
        s2r = row_pool.tile([1, STT], F32, tag="s2r")

        # frames: queries (cols 0:FQ) and supports (cols FQ:F), K on partitions
        qst = []
        for kc in range(KC):
            t = qst_pool.tile([128, F], BF, tag="qst")
            nc.sync.dma_start(t[:, 0:FQ], qt_d.ap()[kc * 128:(kc + 1) * 128, :])
            nc.sync.dma_start(t[:, FQ:F], st_d.ap()[kc * 128:(kc + 1) * 128, :])
            qst.append(t)

        qe_tiles, se_tiles = [], []
        with tc.tile_pool(name="pp", bufs=3, space="PSUM") as pp_pool, \
             tc.tile_pool(name="ps", bufs=3, space="PSUM") as ps_pool, \
             tc.tile_pool(name="o2", bufs=2, space="PSUM") as o_pool:
            for jc in range(JCN):
                # ---- P matmuls for this embed chunk ----
                wtg = []
                for kc in range(KC):
                    wt = wt_pool.tile([128, WG], BF, tag="wt")
                    nc.sync.dma_start(
                        wt[:], w_d.ap()[kc * 128:(kc + 1) * 128,
                                        jc * WG:(jc + 1) * WG])
                    wtg.append(wt)
                psq, pss = [], []
                for a in range(TSS):
                    pq = pp_pool.tile([128, FQ], F32, tag="pp",
                                      name=f"pq_{jc}_{a}")
                    pg = ps_pool.tile([128, FS], F32, tag="ps",
                                      name=f"pg_{jc}_{a}")
                    for kc in range(KC):
                        lhs = wtg[kc][:, a * 128:(a + 1) * 128]
                        nc.tensor.matmul(pq[:], lhs, qst[kc][:, 0:FQ],
                                         start=(kc == 0), stop=(kc == KC - 1))
                        nc.tensor.matmul(pg[:], lhs, qst[kc][:, FQ:F],
                                         start=(kc == 0), stop=(kc == KC - 1))
                    psq.append(pq)
                    pss.append(pg)
                bcol = b_sb[:, jc:jc + 1]
                # a=0 drains (bias folded); support a=1/2 drained for GPSIMD
                p0q = psb_pool.tile([128, FQ], F32, tag="p0q")
                nc.vector.tensor_scalar_add(p0q[:], psq[0][:], bcol)
                p0s = pss_pool.tile([128, FS], F32, tag="p0s")
                nc.vector.tensor_scalar_add(p0s[:], pss[0][:], bcol)
                p1s = pss_pool.tile([128, FS], F32, tag="p1s")
                nc.vector.tensor_copy(p1s[:], pss[1][:])
                p2s = pss_pool.tile([128, FS], F32, tag="p2s")
                nc.vector.tensor_copy(p2s[:], pss[2][:])

                # ---- tuple combine: queries (DVE), t-major columns ----
                # S01[r*50+q] = P0[f0*50+q] + P1[f1*50+q]  (dense [128,50] ops)
                s01q = s01_pool.tile([128, NRUN * QPC], F32, tag="s01q")
                for r, (t0, ln, f0, f1, f2s) in enumerate(RUNS):
                    nc.vector.tensor_add(
                        s01q[:, r * QPC:(r + 1) * QPC],
                        p0q[:, f0 * QPC:(f0 + 1) * QPC],
                        psq[1][:, f1 * QPC:(f1 + 1) * QPC])
                tmpq = tmp_pool.tile([128, QT], BF, tag="tmpq")
                for r, (t0, ln, f0, f1, f2s) in enumerate(RUNS):
                    o = tmpq[:, t0 * QPC:(t0 + ln) * QPC]
                    i0 = s01q[:, r * QPC:(r + 1) * QPC] \
                        .unsqueeze(1).broadcast_to((128, ln, QPC))
                    i1 = psq[2][:, f2s * QPC:(f2s + ln) * QPC] \
                        .rearrange('p (t q) -> p t q', q=QPC)
                    nc.vector.tensor_add(
                        o.rearrange('p (t q) -> p t q', q=QPC), i0, i1)
                qe = qe_pool.tile([128, QTP], BF, tag="qe")
                nc.vector.memset(qe[:, QT:QTP], 0.0)
                nc.scalar.activation(qe[:, 0:QT], tmpq[:], RELU)
                qe_tiles.append(qe)

                # ---- tuple combine: supports (GPSIMD, from drained copies) ----
                s01s = s01_pool.tile([128, NRUN * NS], F32, tag="s01s")
                for r, (t0, ln, f0, f1, f2s) in enumerate(RUNS):
                    nc.gpsimd.tensor_add(
                        s01s[:, r * NS:(r + 1) * NS],
                        p0s[:, f0 * NS:(f0 + 1) * NS],
                        p1s[:, f1 * NS:(f1 + 1) * NS])
                tmps = tmp_pool.tile([128, STT], BF, tag="tmps")
                for r, (t0, ln, f0, f1, f2s) in enumerate(RUNS):
                    o = tmps[:, t0 * NS:(t0 + ln) * NS]
                    i0 = s01s[:, r * NS:(r + 1) * NS] \
                        .unsqueeze(1).broadcast_to((128, ln, NS))
                    i1 = p2s[:, f2s * NS:(f2s + ln) * NS] \
                        .rearrange('p (t s) -> p t s', s=NS)
                    nc.gpsimd.tensor_add(
                        o.rearrange('p (t s) -> p t s', s=NS), i0, i1)
                sett = tmp_pool.tile([128, STT], BF, tag="sett")
                nc.scalar.activation(sett[:], tmps[:], RELU)
                # reorder t-major -> class-major: out[c*280+sh*56+t] = in[t*25+c*5+sh]
                se = se_pool.tile([128, STT], BF, tag="se")
                se_v = se.rearrange('p (c sh t) -> p c sh t', sh=SHOT, t=TN)
                in_v = sett.rearrange('p (t c sh) -> p c sh t', c=WAY, sh=SHOT)
                nc.gpsimd.tensor_copy(se_v, in_v)
                se_tiles.append(se)

                # ---- squared norms: bf16 squares, ones-matmul, f32 row accum ----
                sq = sq_pool.tile([128, QTP], BF, tag="sq")
                nc.vector.tensor_mul(sq[:], qe[:], qe[:])
                sqs = sq_pool.tile([128, STT], BF, tag="sqs")
                nc.gpsimd.tensor_mul(sqs[:], se[:], se[:])
                for lo in range(0, QTP, 512):
                    hi = min(lo + 512, QTP)
                    op = o_pool.tile([1, 512], F32, tag="o", name=f"oq_{jc}_{lo}")
                    nc.tensor.matmul(op[:, 0:hi - lo], ones[:], sq[:, lo:hi],
                                     start=True, stop=True)
                    if jc == 0:
                        nc.vector.tensor_copy(q2r[:, lo:hi], op[:, 0:hi - lo])
                    else:
                        nc.vector.tensor_add(q2r[:, lo:hi], q2r[:, lo:hi],
                                             op[:, 0:hi - lo])
                for lo in range(0, STT, 512):
                    hi = min(lo + 512, STT)
                    op = o_pool.tile([1, 512], F32, tag="o", name=f"os_{jc}_{lo}")
                    nc.tensor.matmul(op[:, 0:hi - lo], ones[:], sqs[:, lo:hi],
                                     start=True, stop=True)
                    if jc == 0:
                        nc.vector.tensor_copy(s2r[:, lo:hi], op[:, 0:hi - lo])
                    else:
                        nc.vector.tensor_add(s2r[:, lo:hi], s2r[:, lo:hi],
                                             op[:, 0:hi - lo])

            # -q2/2 and -s2/2 rows, then assemble the K=2 chunk9 operands
            nc.vector.tensor_scalar_mul(q2n[:], q2r[:], -0.5)
            nc.vector.tensor_scalar_mul(s2n[:], s2r[:], -0.5)
            nc.sync.dma_start(lhsT9[0:1, :], q2n[:])
            nc.sync.dma_start(lhsT9[1:2, :], ones_row[:])
            nc.sync.dma_start(rhs9[0:1, :], ones_row[:, 0:STT])
            nc.sync.dma_start(rhs9[1:2, :], s2n[:])

        # ---- dots + class-min + sqrt + tuple-mean ----
        with tc.tile_pool(name="pd", bufs=7, space="PSUM") as pd_pool, \
             tc.tile_pool(name="pl", bufs=1, space="PSUM") as pl_pool:
            lpsum = pl_pool.tile([WAY, QPC], F32, tag="pl")
            for mt in range(NMT):
                pds = [pd_pool.tile([128, SPC], F32, tag="pd",
                                    name=f"pd_{mt}_{c}") for c in range(WAY)]
                for kc2 in range(JCN):
                    lhs = qe_tiles[kc2][:, mt * 128:(mt + 1) * 128]
                    for c in range(WAY):
                        nc.tensor.matmul(pds[c][:], lhs,
                                         se_tiles[kc2][:, c * SPC:(c + 1) * SPC],
                                         start=(kc2 == 0), stop=False)
                lhs9 = lhsT9[:, mt * 128:(mt + 1) * 128]
                for c in range(WAY):
                    nc.tensor.matmul(pds[c][:], lhs9, rhs9[:, c * SPC:(c + 1) * SPC],
                                     start=False, stop=True)
                mred = eps_pool.tile([128, WAY], F32, tag="mred")
                for c in range(WAY):
                    nc.vector.tensor_reduce(mred[:, c:c + 1], pds[c][:],
                                            axis=mybir.AxisListType.X,
                                            op=mybir.AluOpType.max)
                r1 = eps_pool.tile([128, WAY], F32, tag="r1")
                nc.scalar.activation(r1[:], mred[:], RELU, scale=-2.0)
                dsb = eps_pool.tile([128, WAY], BF, tag="dsb")
                nc.scalar.activation(dsb[:], r1[:], SQRT)
                nc.tensor.matmul(lpsum[:], dsb[:],
                                 bm_sb[:, mt * QPC:(mt + 1) * QPC],
                                 start=(mt == 0), stop=(mt == NMT - 1))
            outsb = misc_pool.tile([WAY, QPC], F32, tag="outsb")
            nc.scalar.mul(outsb[:], lpsum[:], -1.0 / TN)
            nc.sync.dma_start(out_d.ap(), outsb[:])

    nc.compile()
    return nc


_NC = None


def _get_nc():
    global _NC
    if _NC is None:
        _NC = build_nc()
    return _NC


def _host_prep(support_set, support_labels, queries, W, b):
    support_set = np.asarray(support_set)
    support_labels = np.asarray(support_labels)
    queries = np.asarray(queries)
    W = np.asarray(W)
    b = np.asarray(b)

    order = np.argsort(support_labels, kind='stable')
    S = support_set[order]                                        # class-major
    # st[d, f*25+s]
    st = np.ascontiguousarray(
        S.transpose(2, 1, 0).reshape(D, FS)).astype(BF16)
    # W[j, a*2048+d] -> w2[d, jc*384 + a*128 + jj],  j = jc*128+jj
    w2 = np.ascontiguousarray(
        W.reshape(JCN, 128, TSS, D).transpose(3, 0, 2, 1).reshape(D, TSS * D2)
    ).astype(BF16)
    bsb = np.ascontiguousarray(b.reshape(JCN, 128).T).astype(np.float32)
    # qt' = t*50 + q  ->  q = qt' % 50
    bm = np.zeros((128, NMT * QPC), np.float32)
    for g in range(QT):
        mt, p = divmod(g, 128)
        bm[p, mt * QPC + g % QPC] = 1.0
    bmh = bm.astype(BF16)
    in_maps = []
    for c in range(NCORES):
        qs = queries[c * QPC:(c + 1) * QPC]
        # qt[d, f*50+q]
        qtc = np.ascontiguousarray(
            qs.transpose(2, 1, 0).reshape(D, FQ)).astype(BF16)
        in_maps.append({"qt": qtc, "st": st, "w": w2, "b": bsb, "bm": bmh})
    return in_maps


def kernel(support_set, support_labels, queries, W, b):
    in_maps = _host_prep(support_set, support_labels, queries, W, b)
    nc = _get_nc()
    res = run_bass_kernel_spmd(nc, in_maps, core_ids=list(range(NCORES)))
    outs = [np.asarray(res.results[c]["out"]).T for c in range(NCORES)]
    return np.ascontiguousarray(np.concatenate(outs, axis=0)).astype(np.float32)


# revision 14
# speedup vs baseline: 1.0515x; 1.0515x over previous
"""Trainium2 Bass kernel for nn_DistanceLoss (retrieval_knn, 5-way few-shot
temporal-tuple distance logits).

Math (per the reference):
  tuples = C(8,3) = 56 frame triples; embed dim 1024; supports grouped 5/class.
  qe = relu(q_tuples @ W.T + b); se likewise.
  logits[q,c] = -mean_t min_s ||qe[q,t] - se[c,s]||

Key restructuring on device:
  1. The 6144-wide gather-matmul is factorized: P[j,(a,f)] per frame (7x fewer
     FLOPs), then tuple-combine via run-grouped adds.  Frame columns are
     f-major and tuple columns t-major so every combine op streams contiguous
     50-element (query) / 25-element (support) inner runs.
  2. dist^2 = -2*(dot - q2/2 - s2/2): the norm terms are folded into the dots
     matmul as an extra K=2 accumulation chunk, so the class-min is a plain
     reduce_max over each class's PSUM block.
  3. mean over the 56 tuples is a [128,5].T @ [128,50] block-ones matmul.

Sharding: data-parallel over queries (50/core on 8 cores); support set and
weights replicated; host concatenates the per-core [5,50] outputs.
"""
import sys

sys.path.insert(0, '/opt/trn_rl_repo')
import numpy as np
import ml_dtypes
from itertools import combinations
from contextlib import ExitStack

from concourse import bass, bacc, tile, mybir
from concourse.bass_utils import run_bass_kernel_spmd

BF16 = ml_dtypes.bfloat16
F32 = mybir.dt.float32
BF = mybir.dt.bfloat16
RELU = mybir.ActivationFunctionType.Relu
SQRT = mybir.ActivationFunctionType.Sqrt

WAY, TSS = 5, 3
NS, NQ, SEQ, D = 25, 400, 8, 2048
D2 = 1024
NCORES = 8
QPC = NQ // NCORES              # 50 queries per core
SHOT = NS // WAY                # 5
TUP = list(combinations(range(SEQ), TSS))
TN = len(TUP)                   # 56
QT = QPC * TN                   # 2800 query-tuple columns per core
NMT = (QT + 127) // 128         # 22 M-tiles
QTP = NMT * 128                 # 2816 (padded)
STT = NS * TN                   # 1400 support-tuple columns
SPC = STT // WAY                # 280 per class
KC = D // 128                   # 16 contraction chunks for P
JCN = D2 // 128                 # 8 embed-dim chunks
FQ = QPC * SEQ                  # 400 query frame-columns
FS = NS * SEQ                   # 200 support frame-columns
F = FQ + FS                     # 600
WG = TSS * 128                  # 384 W columns per (kc, jc) group
NRUN = None

# runs of consecutive tuples sharing (f0, f1); within a run f2 is consecutive
RUNS = []
_t = 0
while _t < TN:
    f0, f1, f2 = TUP[_t]
    ln = 1
    while _t + ln < TN and TUP[_t + ln][0] == f0 and TUP[_t + ln][1] == f1:
        ln += 1
    RUNS.append((_t, ln, f0, f1, f2))
    _t += ln
NRUN = len(RUNS)                # 21


def build_nc():
    nc = bacc.Bacc("TRN2", target_bir_lowering=False, debug=False)
    # qt: [d, f*50+q], st: [d, f*25+s] (s class-sorted), w: [d, jc*384+a*128+jj]
    qt_d = nc.dram_tensor("qt", [D, FQ], BF, kind="ExternalInput")
    st_d = nc.dram_tensor("st", [D, FS], BF, kind="ExternalInput")
    w_d = nc.dram_tensor("w", [D, TSS * D2], BF, kind="ExternalInput")
    b_d = nc.dram_tensor("b", [128, JCN], F32, kind="ExternalInput")
    bm_d = nc.dram_tensor("bm", [128, NMT * QPC], BF, kind="ExternalInput")
    out_d = nc.dram_tensor("out", [WAY, QPC], F32, kind="ExternalOutput")

    with tile.TileContext(nc) as tc, ExitStack() as ctx:
        ep = ctx.enter_context
        wt_pool = ep(tc.tile_pool(name="wt", bufs=20))
        qst_pool = ep(tc.tile_pool(name="qst", bufs=KC))
        psb_pool = ep(tc.tile_pool(name="psb", bufs=2))
        pss_pool = ep(tc.tile_pool(name="pssb", bufs=4))
        s01_pool = ep(tc.tile_pool(name="s01", bufs=2))
        tmp_pool = ep(tc.tile_pool(name="tmp", bufs=2))
        qe_pool = ep(tc.tile_pool(name="qe", bufs=JCN))
        se_pool = ep(tc.tile_pool(name="se", bufs=JCN))
        sq_pool = ep(tc.tile_pool(name="sq", bufs=2))
        row_pool = ep(tc.tile_pool(name="row", bufs=1))
        misc_pool = ep(tc.tile_pool(name="misc", bufs=1))
        eps_pool = ep(tc.tile_pool(name="eps", bufs=3))

        # constants / persistent rows.
        # chunk9 K=2 operands: lhsT9 row0 = -q2/2 (bf16), row1 = ones;
        # rhs9 row0 = ones, row1 = -s2/2, so it contributes q2n[m] + s2n[n].
        # DVE compute must start at partition 0, so rows are computed in
        # 1-partition tiles and placed into partition 1 via SBUF->SBUF DMA.
        ones = misc_pool.tile([128, 1], BF, tag="ones")
        nc.vector.memset(ones[:], 1.0)
        ones_row = misc_pool.tile([1, QTP], BF, tag="ones_row")
        nc.vector.memset(ones_row[:], 1.0)
        b_sb = misc_pool.tile([128, JCN], F32, tag="bsb")
        nc.sync.dma_start(b_sb[:], b_d.ap())
        bm_sb = misc_pool.tile([128, NMT * QPC], BF, tag="bm")
        nc.sync.dma_start(bm_sb[:], bm_d.ap())
        lhsT9 = row_pool.tile([2, QTP], BF, tag="lhsT9")
        rhs9 = row_pool.tile([2, STT], BF, tag="rhs9")
        q2n = row_pool.tile([1, QTP], BF, tag="q2n")
        s2n = row_pool.tile([1, STT], BF, tag="s2n")
        q2r = row_pool.tile([1, QTP], F32, tag="q2r")
        s2r = row_pool.tile([1, STT], F32, tag="s2r")

        # frames: queries (cols 0:FQ) and supports (cols FQ:F), K on partitions.
        # jc=0's W tiles are interleaved so the first matmuls start early.
        qst = []
        wtg0 = []
        for kc in range(KC):
            wt = wt_pool.tile([128, WG], BF, tag="wt", name=f"wt0_{kc}")
            nc.sync.dma_start(
                wt[:], w_d.ap()[kc * 128:(kc + 1) * 128, 0:WG])
            wtg0.append(wt)
            t = qst_pool.tile([128, F], BF, tag="qst")
            nc.sync.dma_start(t[:, 0:FQ], qt_d.ap()[kc * 128:(kc + 1) * 128, :])
            nc.sync.dma_start(t[:, FQ:F], st_d.ap()[kc * 128:(kc + 1) * 128, :])
            qst.append(t)

        qe_tiles, se_tiles = [], []
        sq_prev = None
        with tc.tile_pool(name="pp", bufs=3, space="PSUM") as pp_pool, \
             tc.tile_pool(name="ps", bufs=3, space="PSUM") as ps_pool, \
             tc.tile_pool(name="o2", bufs=2, space="PSUM") as o_pool:

            def emit_norm_mms(jcx, sq, sqs):
                # Sum(sq) over partitions via ones-matmul, accumulated into
                # f32 rows on DVE.  Emitted one jc late so the PE stream does
                # not stall on the ACT square chain.
                for lo in range(0, QTP, 512):
                    hi = min(lo + 512, QTP)
                    op = o_pool.tile([1, 512], F32, tag="o",
                                     name=f"oq_{jcx}_{lo}")
                    nc.tensor.matmul(op[:, 0:hi - lo], ones[:], sq[:, lo:hi],
                                     start=True, stop=True)
                    if jcx == 0:
                        nc.vector.tensor_copy(q2r[:, lo:hi], op[:, 0:hi - lo])
                    else:
                        nc.vector.tensor_add(q2r[:, lo:hi], q2r[:, lo:hi],
                                             op[:, 0:hi - lo])
                for lo in range(0, STT, 512):
                    hi = min(lo + 512, STT)
                    op = o_pool.tile([1, 512], F32, tag="o",
                                     name=f"os_{jcx}_{lo}")
                    nc.tensor.matmul(op[:, 0:hi - lo], ones[:], sqs[:, lo:hi],
                                     start=True, stop=True)
                    if jcx == 0:
                        nc.vector.tensor_copy(s2r[:, lo:hi], op[:, 0:hi - lo])
                    else:
                        nc.vector.tensor_add(s2r[:, lo:hi], s2r[:, lo:hi],
                                             op[:, 0:hi - lo])

            for jc in range(JCN):
                # ---- P matmuls for this embed chunk ----
                if jc == 0:
                    wtg = wtg0
                else:
                    wtg = []
                    for kc in range(KC):
                        wt = wt_pool.tile([128, WG], BF, tag="wt",
                                          name=f"wt{jc}_{kc}")
                        nc.sync.dma_start(
                            wt[:], w_d.ap()[kc * 128:(kc + 1) * 128,
                                            jc * WG:(jc + 1) * WG])
                        wtg.append(wt)
                psq, pss = [], []
                for a in range(TSS):
                    pq = pp_pool.tile([128, FQ], F32, tag="pp",
                                      name=f"pq_{jc}_{a}")
                    pg = ps_pool.tile([128, FS], F32, tag="ps",
                                      name=f"pg_{jc}_{a}")
                    for kc in range(KC):
                        lhs = wtg[kc][:, a * 128:(a + 1) * 128]
                        nc.tensor.matmul(pq[:], lhs, qst[kc][:, 0:FQ],
                                         start=(kc == 0), stop=(kc == KC - 1))
                        nc.tensor.matmul(pg[:], lhs, qst[kc][:, FQ:F],
                                         start=(kc == 0), stop=(kc == KC - 1))
                    psq.append(pq)
                    pss.append(pg)
                if sq_prev is not None:
                    emit_norm_mms(jc - 1, *sq_prev)
                bcol = b_sb[:, jc:jc + 1]
                # a=0 drains (bias folded); support a=1/2 drained for GPSIMD
                p0q = psb_pool.tile([128, FQ], F32, tag="p0q")
                nc.vector.tensor_scalar_add(p0q[:], psq[0][:], bcol)
                p0s = pss_pool.tile([128, FS], F32, tag="p0s")
                nc.vector.tensor_scalar_add(p0s[:], pss[0][:], bcol)
                p1s = pss_pool.tile([128, FS], F32, tag="p1s")
                nc.vector.tensor_copy(p1s[:], pss[1][:])
                p2s = pss_pool.tile([128, FS], F32, tag="p2s")
                nc.vector.tensor_copy(p2s[:], pss[2][:])

                # ---- tuple combine: queries (DVE), t-major columns ----
                # S01[r*50+q] = P0[f0*50+q] + P1[f1*50+q]  (dense [128,50] ops)
                s01q = s01_pool.tile([128, NRUN * QPC], F32, tag="s01q")
                for r, (t0, ln, f0, f1, f2s) in enumerate(RUNS):
                    nc.vector.tensor_add(
                        s01q[:, r * QPC:(r + 1) * QPC],
                        p0q[:, f0 * QPC:(f0 + 1) * QPC],
                        psq[1][:, f1 * QPC:(f1 + 1) * QPC])
                tmpq = tmp_pool.tile([128, QT], BF, tag="tmpq")
                for r, (t0, ln, f0, f1, f2s) in enumerate(RUNS):
                    o = tmpq[:, t0 * QPC:(t0 + ln) * QPC]
                    i0 = s01q[:, r * QPC:(r + 1) * QPC] \
                        .unsqueeze(1).broadcast_to((128, ln, QPC))
                    i1 = psq[2][:, f2s * QPC:(f2s + ln) * QPC] \
                        .rearrange('p (t q) -> p t q', q=QPC)
                    nc.vector.tensor_add(
                        o.rearrange('p (t q) -> p t q', q=QPC), i0, i1)
                qe = qe_pool.tile([128, QTP], BF, tag="qe")
                nc.vector.memset(qe[:, QT:QTP], 0.0)
                nc.scalar.activation(qe[:, 0:QT], tmpq[:], RELU)
                qe_tiles.append(qe)

                # ---- tuple combine: supports (GPSIMD, from drained copies) ----
                s01s = s01_pool.tile([128, NRUN * NS], F32, tag="s01s")
                for r, (t0, ln, f0, f1, f2s) in enumerate(RUNS):
                    nc.gpsimd.tensor_add(
                        s01s[:, r * NS:(r + 1) * NS],
                        p0s[:, f0 * NS:(f0 + 1) * NS],
                        p1s[:, f1 * NS:(f1 + 1) * NS])
                tmps = tmp_pool.tile([128, STT], BF, tag="tmps")
                for r, (t0, ln, f0, f1, f2s) in enumerate(RUNS):
                    o = tmps[:, t0 * NS:(t0 + ln) * NS]
                    i0 = s01s[:, r * NS:(r + 1) * NS] \
                        .unsqueeze(1).broadcast_to((128, ln, NS))
                    i1 = p2s[:, f2s * NS:(f2s + ln) * NS] \
                        .rearrange('p (t s) -> p t s', s=NS)
                    nc.gpsimd.tensor_add(
                        o.rearrange('p (t s) -> p t s', s=NS), i0, i1)
                sett = tmp_pool.tile([128, STT], BF, tag="sett")
                nc.scalar.activation(sett[:], tmps[:], RELU)
                # reorder t-major -> class-major: out[c*280+sh*56+t] = in[t*25+c*5+sh]
                se = se_pool.tile([128, STT], BF, tag="se")
                se_v = se.rearrange('p (c sh t) -> p c sh t', sh=SHOT, t=TN)
                in_v = sett.rearrange('p (t c sh) -> p c sh t', c=WAY, sh=SHOT)
                nc.gpsimd.tensor_copy(se_v, in_v)
                se_tiles.append(se)

                # ---- squared norms on ACT; ones-matmuls deferred one jc ----
                sq = sq_pool.tile([128, QTP], BF, tag="sq")
                nc.scalar.square(sq[:], qe[:])
                sqs = sq_pool.tile([128, STT], BF, tag="sqs")
                nc.scalar.square(sqs[:], se[:])
                sq_prev = (sq, sqs)

            emit_norm_mms(JCN - 1, *sq_prev)

            # -q2/2 and -s2/2 rows, then assemble the K=2 chunk9 operands
            nc.vector.tensor_scalar_mul(q2n[:], q2r[:], -0.5)
            nc.vector.tensor_scalar_mul(s2n[:], s2r[:], -0.5)
            nc.sync.dma_start(lhsT9[0:1, :], q2n[:])
            nc.sync.dma_start(lhsT9[1:2, :], ones_row[:])
            nc.sync.dma_start(rhs9[0:1, :], ones_row[:, 0:STT])
            nc.sync.dma_start(rhs9[1:2, :], s2n[:])

        # ---- dots + class-min + sqrt + tuple-mean ----
        with tc.tile_pool(name="pd", bufs=7, space="PSUM") as pd_pool, \
             tc.tile_pool(name="pl", bufs=1, space="PSUM") as pl_pool, \
             tc.tile_pool(name="dsb", bufs=NMT) as dsb_pool:
            dsb_tiles = []
            for mt in range(NMT):
                pds = [pd_pool.tile([128, SPC], F32, tag="pd",
                                    name=f"pd_{mt}_{c}") for c in range(WAY)]
                for kc2 in range(JCN):
                    lhs = qe_tiles[kc2][:, mt * 128:(mt + 1) * 128]
                    for c in range(WAY):
                        nc.tensor.matmul(pds[c][:], lhs,
                                         se_tiles[kc2][:, c * SPC:(c + 1) * SPC],
                                         start=(kc2 == 0), stop=False)
                lhs9 = lhsT9[:, mt * 128:(mt + 1) * 128]
                for c in range(WAY):
                    nc.tensor.matmul(pds[c][:], lhs9, rhs9[:, c * SPC:(c + 1) * SPC],
                                     start=False, stop=True)
                mred = eps_pool.tile([128, WAY], F32, tag="mred")
                for c in range(WAY):
                    nc.vector.tensor_reduce(mred[:, c:c + 1], pds[c][:],
                                            axis=mybir.AxisListType.X,
                                            op=mybir.AluOpType.max)
                r1 = eps_pool.tile([128, WAY], F32, tag="r1")
                nc.scalar.activation(r1[:], mred[:], RELU, scale=-2.0)
                dsb = dsb_pool.tile([128, WAY], BF, tag="dsb",
                                    name=f"dsb_{mt}")
                nc.scalar.activation(dsb[:], r1[:], SQRT)
                dsb_tiles.append(dsb)
            # tuple-mean at the end so the PE stream never waits on ACT
            lpsum = pl_pool.tile([WAY, QPC], F32, tag="pl")
            for mt in range(NMT):
                nc.tensor.matmul(lpsum[:], dsb_tiles[mt][:],
                                 bm_sb[:, mt * QPC:(mt + 1) * QPC],
                                 start=(mt == 0), stop=(mt == NMT - 1))
            outsb = misc_pool.tile([WAY, QPC], F32, tag="outsb")
            nc.scalar.mul(outsb[:], lpsum[:], -1.0 / TN)
            nc.sync.dma_start(out_d.ap(), outsb[:])

    nc.compile()
    return nc


_NC = None


def _get_nc():
    global _NC
    if _NC is None:
        _NC = build_nc()
    return _NC


def _host_prep(support_set, support_labels, queries, W, b):
    support_set = np.asarray(support_set)
    support_labels = np.asarray(support_labels)
    queries = np.asarray(queries)
    W = np.asarray(W)
    b = np.asarray(b)

    order = np.argsort(support_labels, kind='stable')
    S = support_set[order]                                        # class-major
    # st[d, f*25+s]
    st = np.ascontiguousarray(
        S.transpose(2, 1, 0).reshape(D, FS)).astype(BF16)
    # W[j, a*2048+d] -> w2[d, jc*384 + a*128 + jj],  j = jc*128+jj
    w2 = np.ascontiguousarray(
        W.reshape(JCN, 128, TSS, D).transpose(3, 0, 2, 1).reshape(D, TSS * D2)
    ).astype(BF16)
    bsb = np.ascontiguousarray(b.reshape(JCN, 128).T).astype(np.float32)
    # qt' = t*50 + q  ->  q = qt' % 50
    bm = np.zeros((128, NMT * QPC), np.float32)
    for g in range(QT):
        mt, p = divmod(g, 128)
        bm[p, mt * QPC + g % QPC] = 1.0
    bmh = bm.astype(BF16)
    in_maps = []
    for c in range(NCORES):
        qs = queries[c * QPC:(c + 1) * QPC]
        # qt[d, f*50+q]
        qtc = np.ascontiguousarray(
            qs.transpose(2, 1, 0).reshape(D, FQ)).astype(BF16)
        in_maps.append({"qt": qtc, "st": st, "w": w2, "b": bsb, "bm": bmh})
    return in_maps


def kernel(support_set, support_labels, queries, W, b):
    in_maps = _host_prep(support_set, support_labels, queries, W, b)
    nc = _get_nc()
    res = run_bass_kernel_spmd(nc, in_maps, core_ids=list(range(NCORES)))
    outs = [np.asarray(res.results[c]["out"]).T for c in range(NCORES)]
    return np.ascontiguousarray(np.concatenate(outs, axis=0)).astype(np.float32)
